# revision 2
# baseline (speedup 1.0000x reference)
"""DGCNN (nn_DGCNN_43911745634410) Trainium2 kernel.

The only heavy compute is xw = x @ gcn1_W with x [129, 262144] f32 (~135 MB)
and gcn1_W [262144, 1] — a memory-bound matvec shared by all three edge-attr
channels.  Everything downstream (segment-sums over 16K edges, a 129-element
sort, two tiny conv1ds, three FCs) is a few hundred KFLOPs and runs on the
host in f64.

Device strategy (8 NeuronCores, tensor-parallel over the feature dim F):
  - core c gets x[:, c*32768:(c+1)*32768] uploaded in *fp16* (8.5 MB/core,
    half the f32 bytes — DMA is the roofline at 360 GB/s/core).  fp16
    quantization of x and w changes the final [1,2] output by ~5e-3 relative,
    well inside the 2e-2 gate (verified against the exact inputs).
  - layout puts 128 feature-slots on SBUF partitions and (node, r) along the
    free dim, so the DVE's tensor_tensor (the only fp16-2x-rate op) does the
    x*w multiply with w broadcast along the node axis via a stride-0 AP dim,
    and the PE reduces across partitions with a ones-column matmul
    accumulating into PSUM (ldweights/matmul free-dim cost only).
  - 31 "r8" tiles [128, 129*8] (1024 feats each) + 4 "r2" tiles [128, 129*2]
    (256 feats) cover the 32768-feature shard.  One DMA per tile keeps the
    DVE 900ns-sem pipeline full; the tiny r2 tiles shorten the tail.
  - tiles 0..26 accumulate into psum accm [1,1032] (3 bank-aligned flat
    matmuls per tile, stopped early so the wide psum->sbuf copy hides under
    the stream); tiles 27..30 (r-pair-sliced matmuls) and the r2 tiles
    accumulate into acc2 [1,258] whose short copy sits in the tail.
  - host folds the [1,1290] partials (f64), all-reduces the 8 cores, and
    runs the exact reference downstream in f64.
"""
from contextlib import ExitStack

import numpy as np

import concourse.bass as bass
from concourse import mybir
from concourse.bass_utils import run_bass_kernel_spmd

F32 = mybir.dt.float32
F16 = mybir.dt.float16

N = 129
F = 262144
NCORES = 8
SH = F // NCORES          # 32768 features per core
NT8 = 31                  # r8 tiles: 128 partitions x 8 feats = 1024 feats
NT2 = 4                   # r2 tiles: 128 partitions x 2 feats = 256 feats
T8FREE = N * 8            # 1032
T2FREE = N * 2            # 258
WCOLS = 257               # 248 r8 w-cols + 8 r2 w-cols + ones column
NACCM = 27                # r8 tiles 0..26 -> accm; 27..30 -> acc2 (sliced)
OUTW = 1032 + 258         # 1290

# DMA chunks: [w-block + tile0], [tile1] .. [tile30], [r2 x3], [r2 x1]
CHUNK_TILES = (
    [[("r8", 0)]]
    + [[("r8", t)] for t in range(1, NT8)]
    + [[("r2", 0), ("r2", 1), ("r2", 2)], [("r2", 3)]]
)

_NC_CACHE = None


def _build_matvec_bass():
    nc = bass.Bass("TRN2")
    total = 128 * (WCOLS + NT8 * T8FREE + NT2 * T2FREE)
    xd = nc.dram_tensor("x_s", [total], F16, kind="ExternalInput")
    out = nc.dram_tensor("part", [1, OUTW], F32, kind="ExternalOutput")

    with ExitStack() as ctx:
        bufs = []
        for i, tl in enumerate(CHUNK_TILES):
            n = sum(T8FREE if k == "r8" else T2FREE for k, _ in tl)
            if i == 0:
                n += WCOLS
            bufs.append(ctx.enter_context(nc.sbuf_tensor(f"xb{i}", [128, n], F16)))
        res = ctx.enter_context(nc.sbuf_tensor("res", [1, OUTW], F32))
        accm = ctx.enter_context(nc.psum_tensor("accm", [1, 1032], F32))
        acc2 = ctx.enter_context(nc.psum_tensor("acc2", [1, 258], F32))
        x_sems = [
            ctx.enter_context(nc.semaphore(f"x_sem{c}"))
            for c in range(len(CHUNK_TILES))
        ]
        dve_sem = ctx.enter_context(nc.semaphore("dve_sem"))
        pem_sem = ctx.enter_context(nc.semaphore("pem_sem"))
        pe2_sem = ctx.enter_context(nc.semaphore("pe2_sem"))
        act_sem = ctx.enter_context(nc.semaphore("act_sem"))
        out_sem = ctx.enter_context(nc.semaphore("out_sem"))
        block = ctx.enter_context(nc.Block())

        wsb = bufs[0]  # w block lives in cols 0..256 of chunk-0's buffer

        @block.sync
        def _(sync):
            off = 0
            for c, tl in enumerate(CHUNK_TILES):
                n = sum(T8FREE if k == "r8" else T2FREE for k, _ in tl)
                if c == 0:
                    n += WCOLS
                sync.dma_start(
                    bufs[c][:, :],
                    xd[off : off + 128 * n].rearrange("(p f) -> p f", f=n),
                ).then_inc(x_sems[c], 16)
                off += 128 * n
            sync.wait_ge(act_sem, 2)
            sync.dma_start(out[:, :], res[:, :]).then_inc(out_sem, 16)

        # flat tile list with per-buffer offsets
        tiles = []
        for c, tl in enumerate(CHUNK_TILES):
            off = WCOLS if c == 0 else 0
            for k, idx in tl:
                tiles.append((c, off, k, idx))
                off += T8FREE if k == "r8" else T2FREE

        @block.vector
        def _(vector):
            cur = -1
            for (c, off, k, idx) in tiles:
                if c != cur:
                    vector.wait_ge(x_sems[c], 16)
                    cur = c
                if k == "r8":
                    fr, r, wofs = T8FREE, 8, idx * 8
                else:
                    fr, r, wofs = T2FREE, 2, 248 + idx * 2
                xt = bufs[c][:, off : off + fr].rearrange("p (n r) -> p n r", r=r)
                w_ap = (
                    wsb[:, wofs : wofs + r]
                    .unsqueeze(1)
                    .broadcast_to((128, N, r))
                )
                nc.vector.tensor_tensor(
                    xt, xt, w_ap, op=mybir.AluOpType.mult
                ).then_inc(dve_sem, 1)

        @block.tensor
        def _(tensor):
            ones = wsb[:, 256:257]
            n2 = 0
            for ti, (c, off, k, idx) in enumerate(tiles):
                tensor.wait_ge(dve_sem, ti + 1)
                if k == "r8" and idx < NACCM:
                    for (a, b) in ((0, 512), (512, 1024), (1024, 1032)):
                        mm = nc.tensor.matmul(
                            accm[:, a:b],
                            ones,
                            bufs[c][:, off + a : off + b],
                            start=(idx == 0),
                            stop=(idx == NACCM - 1),
                        )
                        if idx == NACCM - 1 and b == 1032:
                            mm.then_inc(pem_sem, 1)
                elif k == "r8":
                    tile = bufs[c][:, off : off + T8FREE].rearrange(
                        "p (n r) -> p n r", r=8
                    )
                    for rp in range(4):
                        nc.tensor.matmul(
                            acc2[:, :].rearrange("p (n r) -> p n r", r=2),
                            ones,
                            tile[:, :, 2 * rp : 2 * rp + 2],
                            start=(n2 == 0),
                            stop=False,
                        )
                        n2 += 1
                else:
                    last = idx == NT2 - 1
                    mm = nc.tensor.matmul(
                        acc2[:, :],
                        ones,
                        bufs[c][:, off : off + T2FREE],
                        start=(n2 == 0),
                        stop=last,
                    )
                    if last:
                        mm.then_inc(pe2_sem, 1)
                    n2 += 1

        @block.scalar
        def _(scalar):
            scalar.wait_ge(pem_sem, 1)
            nc.scalar.copy(res[:, 0:1032], accm[:, :]).then_inc(act_sem, 1)
            scalar.wait_ge(pe2_sem, 1)
            nc.scalar.copy(res[:, 1032:OUTW], acc2[:, :]).then_inc(act_sem, 1)

    return nc


def get_matvec_bass():
    global _NC_CACHE
    if _NC_CACHE is None:
        _NC_CACHE = _build_matvec_bass()
    return _NC_CACHE


def _make_core_inputs(x_np, w_np, core):
    """Host-side fp16 layout prep for one core's shard."""
    xs = x_np[:, core * SH : (core + 1) * SH].astype(np.float16)  # [129, SH]
    ws = w_np[core * SH : (core + 1) * SH].astype(np.float16)

    # r8 tiles: [31, 128, 129, 8]; r2 tiles: [4, 128, 129, 2]
    a8 = np.ascontiguousarray(
        xs[:, : NT8 * 1024].reshape(N, NT8, 128, 8).transpose(1, 2, 0, 3)
    )
    a2 = np.ascontiguousarray(
        xs[:, NT8 * 1024 :].reshape(N, NT2, 128, 2).transpose(1, 2, 0, 3)
    )
    wblk = np.empty((128, WCOLS), np.float16)
    wblk[:, 0:248] = ws[: NT8 * 1024].reshape(NT8, 128, 8).transpose(1, 0, 2).reshape(128, 248)
    wblk[:, 248:256] = ws[NT8 * 1024 :].reshape(NT2, 128, 2).transpose(1, 0, 2).reshape(128, 8)
    wblk[:, 256] = np.float16(1.0)

    parts = []
    i8 = 0
    i2 = 0
    for c, tl in enumerate(CHUNK_TILES):
        cols = []
        if c == 0:
            cols.append(wblk)
        for k, _ in tl:
            if k == "r8":
                cols.append(a8[i8].reshape(128, T8FREE))
                i8 += 1
            else:
                cols.append(a2[i2].reshape(128, T2FREE))
                i2 += 1
        parts.append(np.concatenate(cols, axis=1))
    flat = np.concatenate([p.reshape(-1) for p in parts])
    return {"x_s": np.ascontiguousarray(flat)}


def _reduce_parts(parts):
    """parts: 8 arrays [1, 1290] f32 -> xw [N] f64."""
    xw = np.zeros(N, np.float64)
    for part in parts:
        p = np.asarray(part, np.float64).reshape(-1)
        xw += p[0:1032].reshape(N, 8).sum(1)
        xw += p[1032:OUTW].reshape(N, 2).sum(1)
    return xw


def _matvec_device(x_np, w_np):
    """x [N, F] f32, w [F] f32 -> xw [N] f64 via the 8-core bass kernel."""
    global _NC_CACHE
    in_maps = [_make_core_inputs(x_np, w_np, c) for c in range(NCORES)]
    last_exc = None
    for attempt in range(2):
        try:
            nc = get_matvec_bass()
            res = run_bass_kernel_spmd(nc, in_maps, core_ids=list(range(NCORES)))
            return _reduce_parts([res.results[c]["part"] for c in range(NCORES)])
        except Exception as e:  # transient NRT_EXEC_UNIT_UNRECOVERABLE seen once
            import sys

            print(f"kernel: device run attempt {attempt} failed: {e!r:.200}",
                  file=sys.stderr)
            last_exc = e
            _NC_CACHE = None
    # Last-resort host fallback so a transient device failure still yields a
    # correct result (same fp16 quantization as the device path).
    import sys

    print(f"kernel: device path failed twice ({last_exc!r:.200}); "
          "falling back to host matvec", file=sys.stderr)
    x16 = x_np.astype(np.float16).astype(np.float32)
    w16 = w_np.astype(np.float16).astype(np.float32)
    prod = (x16 * w16[None, :]).astype(np.float16).astype(np.float64)
    return prod.sum(axis=1)


def _downstream(xw, inputs):
    """Everything after xw = x @ gcn1_W, in f64 numpy. Returns [1, 2] f32."""
    edge_index = np.asarray(inputs["edge_index"]).astype(np.int64)
    row, col = edge_index[0], edge_index[1]
    edge_attr = np.asarray(inputs["edge_attr"], np.float64)
    g1b = np.asarray(inputs["gcn1_b"], np.float64)
    g2W = np.asarray(inputs["gcn2_W"], np.float64)
    g2b = np.asarray(inputs["gcn2_b"], np.float64)
    c1w = np.asarray(inputs["conv1_w"], np.float64)
    c1b = np.asarray(inputs["conv1_b"], np.float64)
    c2w = np.asarray(inputs["conv2_w"], np.float64)
    c2b = np.asarray(inputs["conv2_b"], np.float64)
    f1W = np.asarray(inputs["fc1_W"], np.float64)
    f1b = np.asarray(inputs["fc1_b"], np.float64)
    f2W = np.asarray(inputs["fc2_W"], np.float64)
    f2b = np.asarray(inputs["fc2_b"], np.float64)
    f3W = np.asarray(inputs["fc3_W"], np.float64)
    f3b = np.asarray(inputs["fc3_b"], np.float64)

    n = N
    loop = np.arange(n)
    row2 = np.concatenate([row, loop])
    col2 = np.concatenate([col, loop])

    def gcn(xw_vec, ew):
        # PyG GCNConv with edge weights: self-loops (weight 1), symmetric norm.
        ew2 = np.concatenate([ew, np.ones(n)])
        deg = np.zeros(n)
        np.add.at(deg, col2, ew2)
        dinv = np.where(deg > 0, deg**-0.5, 0.0)
        norm = dinv[row2] * ew2 * dinv[col2]
        out = np.zeros(n)
        np.add.at(out, col2, norm * xw_vec[row2])
        return out

    outs = []
    for c in range(3):
        ew = edge_attr[:, c]
        h1 = gcn(xw, ew) + g1b[0]
        h2 = gcn(h1 * g2W[0, 0], ew) + g2b[0]
        # SortPool: jnp.argsort(-h2) is a stable ascending sort of the negation
        perm = np.argsort(-h2, kind="stable")
        hs = np.stack([h1[perm], h2[perm]], axis=1)  # [n, 2]
        z = hs.T  # [2, n]
        L = z.shape[1] - 2
        z1 = np.zeros((3, L))
        for o in range(3):
            for i in range(2):
                for k in range(3):
                    z1[o] += c1w[o, i, k] * z[i, k : k + L]
            z1[o] += c1b[o]
        z1p = np.max(np.stack([z1[:, 0 : L - 2], z1[:, 1 : L - 1], z1[:, 2:L]], 0), 0)
        L2 = z1p.shape[1] - 2
        z2 = np.zeros((1, L2))
        for i in range(3):
            for k in range(3):
                z2[0] += c2w[0, i, k] * z1p[i, k : k + L2]
        z2[0] += c2b[0]
        z2p = np.max(
            np.stack([z2[:, 0 : L2 - 2], z2[:, 1 : L2 - 1], z2[:, 2:L2]], 0), 0
        )
        outs.append(z2p)  # [1, 121]

    allx = np.concatenate(outs, axis=0)  # [3, 121]
    h = allx.reshape(1, -1)

    def elu(v):
        return np.where(v > 0, v, np.expm1(v))

    h = elu(h @ f1W + f1b)
    h = elu(h @ f2W + f2b)
    out = h @ f3W + f3b
    return out.astype(np.float32)


def kernel(**inputs) -> np.ndarray:
    x = np.ascontiguousarray(np.asarray(inputs["x"], np.float32))
    w = np.asarray(inputs["gcn1_W"], np.float32).reshape(-1)
    xw = _matvec_device(x, w)
    return _downstream(xw, inputs)
